# revision 37
# baseline (speedup 1.0000x reference)
"""Trainium2 Bass kernel for nn_MultiHeadDotProductAttention (b=4, L=2048,
d_model=1024, 16 heads x 64 head_dim, additive attention bias, softmax).

Sharding: 8 cores = 2 batch-groups (2 batches each) x 4 head-groups (4 heads
each). Each core computes, for its 2 batches and 4 heads, the full attention
pipeline and an output-projection PARTIAL (summed over its 4 heads); the host
sums the 4 head-group partials per batch and adds the output bias.

v2 design (v1 baseline: 676 us, v2.0: 522 us):
- exp(logits + bias) computed as exp(logits) * exp(bias) with exp(bias)
  precomputed on the HOST (bf16): ACT exps straight out of PSUM, the
  elementwise bias application is a cheap bf16 2x-mode multiply.
- logits (K=64) head-pairs row-tiled (tile_position (0,0)/(64,0)) -> the two
  matmuls run concurrently; both land in one 2-bank PSUM mega-tile
  [128, 2, 512] exp'd by a single ACT instruction.
- AV col-tiled: ctx(hl=0) -> psum rows 0:64 at (0,0), ctx(hl=1) -> rows
  64:128 at (0,64), concurrent. Softmax denominators via M=1 matmuls with a
  ones lhsT col-tiled to rows {0,32,64,96} of one den bank (4 concurrent).
- PSUM banks: lg 2x2 + av 2 + den 1 + aux 1 = 8. Since accumulation groups
  sharing a bank can't each use start=True (first_mm clears the whole bank's
  has_written bits), av/den banks are pre-cleared with a zero-weight K=1
  dummy matmul; the real matmuls use start=False (overwrite-where-unset).
- normalization: one full-height copy den_ps->rs, then per-batch standard
  128x128 matmuls with one-hot-row selector weights broadcast each den row
  to its 64-col half (tiled matmuls cannot write PSUM partitions 64-127:
  s3d3_mm_valid_dst_partition), reciprocal on DVE, one [128,512] multiply
  into ctxT.
- software pipelining: AV/den matmuls for chunk i are emitted at i+AVD so
  the previous block's normalization (urgent fillers) completes before the
  av/den banks are recycled; P1(pair1) runs as filler work inside pair-0
  attention blocks (phase B), P3 out-projection chunks as fillers inside
  pair-1 blocks (phase C). One of the two per-i eb-multiplies runs on
  GPSIMD to unload the DVE.
- output partials written in bf16.
"""

import numpy as np
from contextlib import ExitStack

import ml_dtypes

import concourse.bass as bass
import concourse.mybir as mybir
import concourse.tile as tile
from concourse import bacc
from concourse import bass_utils

F32 = mybir.dt.float32
F32R = mybir.dt.float32r
BF16 = mybir.dt.bfloat16
AF = mybir.ActivationFunctionType

# ---- problem constants (hardcoded per contract) ----
B, L, D = 4, 2048, 1024
H, DH = 16, 64
NB = 2          # batch groups (batches per core = B // NB = 2)
NH = 4          # head groups  (heads per core = H // NH = 4)
BPC = B // NB   # 2 batches per core
HPC = H // NH   # 4 heads per core
PAIRS = HPC // 2
KSUB = D // 128          # 8 contraction subtiles for projections
LCH = 512                # x-stream chunk width (free dim of projection mms)
NLC = L // LCH           # 4 chunks
NQ = 4                   # lq chunks of 512 for attention
NI = 16                  # lk chunks of 128
HD = HPC * DH            # 256 local head dims
HDC = HD // 128          # 2 local hd chunks (= PAIRS)
AVD = 4                  # AV emission delay (software pipeline depth, in i)
GPS_MUL = True           # run one of the two per-i eb-muls on GPSIMD

_CACHED = {}


def _build_bass():
    nc = bacc.Bacc("TRN2", target_bir_lowering=False, debug=False, num_devices=8)

    # ---- DRAM I/O (per core) ----
    xq_d = nc.dram_tensor("xq_t", [BPC, D, L], BF16, kind="ExternalInput")
    xk_d = nc.dram_tensor("xk_t", [BPC, D, L], BF16, kind="ExternalInput")
    eb_d = nc.dram_tensor("eb_t", [HPC, L, L], BF16, kind="ExternalInput")
    wq_d = nc.dram_tensor("wq", [D, HD], BF16, kind="ExternalInput")
    wk_d = nc.dram_tensor("wk", [D, HD], BF16, kind="ExternalInput")
    wv_d = nc.dram_tensor("wv", [D, HD], BF16, kind="ExternalInput")
    wo_d = nc.dram_tensor("wo", [HD, D], BF16, kind="ExternalInput")
    bq_d = nc.dram_tensor("bq", [HD], F32, kind="ExternalInput")
    bk_d = nc.dram_tensor("bk", [HD], F32, kind="ExternalInput")
    bv_d = nc.dram_tensor("bv", [HD], BF16, kind="ExternalInput")
    out_d = nc.dram_tensor("out_part", [BPC, L, D], BF16, kind="ExternalOutput")

    with tile.TileContext(nc) as tc, ExitStack() as top:
        # ---- persistent SBUF ----
        pers = top.enter_context(tc.tile_pool(name="pers", bufs=1))
        qT = pers.tile([128, HDC, BPC, L], BF16)
        kT = pers.tile([128, HDC, BPC, L], BF16)
        v = pers.tile([128, NI, BPC, HPC, DH], BF16)
        ctxT = pers.tile([128, HDC, BPC, L], BF16)
        wo_s = pers.tile([128, HDC, D], BF16)
        wq_s = pers.tile([128, KSUB, HD], BF16)
        wk_s = pers.tile([128, KSUB, HD], BF16)
        wv_s = pers.tile([128, KSUB, HD], BF16)
        bq_s = pers.tile([128, HDC], F32)
        bk_s = pers.tile([128, HDC], F32)
        bv_row = pers.tile([1, HD], BF16)
        ones_m1 = pers.tile([128, 1], BF16)     # den lhsT (K=128, M=1)
        # rep lhsT: one-hot-row selectors, standard 128x128 matmul per batch
        # (tiled matmuls cannot write PSUM partitions 64-127; f32r also
        # breaks col-tiling via the FP32-HI weight path). sel8[:, bb, :]
        # has 1 at (row 32*(2bb+hl), cols hl*64:(hl+1)*64).
        sel8 = pers.tile([128, 2, 128], F32R)
        rs = pers.tile([128, 512], F32R)        # den staging (rows 32j live)
        ones_r1 = pers.tile([1, 128], BF16)     # v-bias lhsT (K=1, M=128)
        zero_w = pers.tile([1, 128], BF16)      # dummy-clear lhsT
        zrow = pers.tile([1, 512], BF16)        # dummy-clear rhs

        nc.sync.dma_start(wo_s[:], wo_d.rearrange("(c p) n -> p c n", p=128))
        nc.sync.dma_start(wq_s[:], wq_d.rearrange("(k p) n -> p k n", p=128))
        nc.sync.dma_start(wk_s[:], wk_d.rearrange("(k p) n -> p k n", p=128))
        nc.sync.dma_start(wv_s[:], wv_d.rearrange("(k p) n -> p k n", p=128))
        nc.sync.dma_start(bq_s[:], bq_d.rearrange("(c p) -> p c", p=128))
        nc.sync.dma_start(bk_s[:], bk_d.rearrange("(c p) -> p c", p=128))
        nc.sync.dma_start(bv_row[:], bv_d[None, :])
        nc.vector.memset(ones_m1[:], 1.0)
        # memset can't write f32r: stage in f32 and copy
        stage8 = pers.tile([128, 2, 128], F32)
        nc.vector.memset(stage8[:], 0.0)
        for bb in range(2):
            for hl in range(2):
                r = 32 * (2 * bb + hl)
                nc.vector.memset(
                    stage8[r:r + 1, bb, hl * 64:(hl + 1) * 64], 1.0
                )
        nc.vector.tensor_copy(sel8[:], stage8[:])
        nc.vector.memset(ones_r1[:], 1.0)
        nc.vector.memset(zero_w[:], 0.0)
        nc.vector.memset(zrow[:], 0.0)
        # prewarm the ACT exp table set during P1
        warm = pers.tile([1, 8], BF16)
        warmi = pers.tile([1, 8], F32)
        nc.vector.memset(warmi[:], 0.0)
        nc.scalar.activation(warm[:], warmi[:], AF.Exp)

        def p1_chunk_ops(pair, b, c, xq_t, xk_t, pspool, pstag, dma=True):
            """Returns [q_closure, k_closure, v0_closure, v1_closure] for one
            (pair, b, c) projection chunk. x tiles are DMA'd by the q/k
            closures; the v closures reuse xk_t."""
            sl = slice(c * LCH, (c + 1) * LCH)
            msl = slice(pair * 128, (pair + 1) * 128)

            stq, stk = {}, {}

            def fq_mm():
                if dma:
                    nc.sync.dma_start(
                        xq_t[:],
                        xq_d[b].rearrange("(k p) l -> p k l", p=128)[:, :, sl],
                    )
                ps = pspool.tile([128, LCH], F32, tag=pstag, name="p1q")
                for k in range(KSUB):
                    nc.tensor.matmul(
                        ps[:], wq_s[:, k, msl], xq_t[:, k, :],
                        start=(k == 0), stop=(k == KSUB - 1),
                    )
                stq["ps"] = ps

            def fq_ev():
                nc.vector.tensor_scalar_add(
                    qT[:, pair, b, sl], stq["ps"][:], bq_s[:, pair:pair + 1]
                )

            def fk_mm():
                if dma:
                    nc.sync.dma_start(
                        xk_t[:],
                        xk_d[b].rearrange("(k p) l -> p k l", p=128)[:, :, sl],
                    )
                ps = pspool.tile([128, LCH], F32, tag=pstag, name="p1k")
                for k in range(KSUB):
                    nc.tensor.matmul(
                        ps[:], wk_s[:, k, msl], xk_t[:, k, :],
                        start=(k == 0), stop=(k == KSUB - 1),
                    )
                stk["ps"] = ps

            def fk_ev():
                nc.vector.tensor_scalar_add(
                    kT[:, pair, b, sl], stk["ps"][:], bk_s[:, pair:pair + 1]
                )

            def mkv(s):
                si = c * (LCH // 128) + s
                st = {}

                def fv_mm():
                    pv = pspool.tile([128, 128], F32, tag=pstag, name="p1v")
                    for k in range(KSUB):
                        nc.tensor.matmul(
                            pv[:], xk_t[:, k, s * 128:(s + 1) * 128],
                            wv_s[:, k, msl],
                            start=(k == 0), stop=False,
                        )
                    nc.tensor.matmul(
                        pv[:], ones_r1[:], bv_row[:, msl],
                        start=False, stop=True,
                    )
                    st["pv"] = pv

                def fv_ev():
                    nc.vector.tensor_copy(
                        v[:, si, b, 2 * pair:2 * pair + 2, :],
                        st["pv"][:].rearrange("p (h d) -> p h d", h=2),
                    )

                return [fv_mm, fv_ev]

            ops = [fq_mm, fq_ev, fk_mm, fk_ev]
            for s in range(LCH // 128):
                ops.extend(mkv(s))
            return ops

        # ---- phase A: projections for pair 0 (dedicated psum, deep bufs) ----
        with ExitStack() as p1:
            xpool = p1.enter_context(tc.tile_pool(name="xs", bufs=2))
            psa = p1.enter_context(tc.tile_pool(name="psa", bufs=4, space="PSUM"))
            for b in range(BPC):
                for c in range(NLC):
                    xq_t = xpool.tile([128, KSUB, LCH], BF16, tag="xq")
                    xk_t = xpool.tile([128, KSUB, LCH], BF16, tag="xk")
                    for f in p1_chunk_ops(0, b, c, xq_t, xk_t, psa, "ps"):
                        f()

        # ---- phases B/C: attention blocks with interleaved fillers ----
        with ExitStack() as p2:
            xpool2 = p2.enter_context(tc.tile_pool(name="xs2", bufs=2))
            ebpool = p2.enter_context(tc.tile_pool(name="ebb", bufs=6))
            etpool = p2.enter_context(tc.tile_pool(name="etb", bufs=6))
            et2pool = p2.enter_context(
                tc.tile_pool(name="et2b", bufs=2 * (AVD + 2))
            )
            reppool = p2.enter_context(tc.tile_pool(name="repb", bufs=2))
            avsbpool = p2.enter_context(tc.tile_pool(name="avsbp", bufs=4))
            opool = p2.enter_context(tc.tile_pool(name="outb", bufs=4))
            lgpool = p2.enter_context(tc.tile_pool(name="lgp", bufs=2, space="PSUM"))
            avpool = p2.enter_context(tc.tile_pool(name="avp", bufs=2, space="PSUM"))
            denpool = p2.enter_context(tc.tile_pool(name="denp", bufs=1, space="PSUM"))
            auxpool = p2.enter_context(tc.tile_pool(name="auxp", bufs=1, space="PSUM"))

            def make_evac_fillers(avs, den_ps, avsb):
                """URGENT: evacuate av psum -> SBUF and den psum -> rs.
                These release the av/den banks so the next block's
                dummy-clears (at i=AVD) don't wait on normalization."""
                fillers = []

                def mkav(bb):
                    def f():
                        nc.vector.tensor_copy(avsb[bb][:], avs[bb][:])
                    return f

                for bb in range(BPC):
                    fillers.append(mkav(bb))

                def cpall():
                    # one full-height copy: non-den rows carry garbage but
                    # sel8's zero rows null them in the rep contraction
                    nc.vector.tensor_copy(rs[:], den_ps[:])
                fillers.append(cpall)
                return fillers

            def make_norm_fillers(p, n, avsb):
                """RELAXED: den replicate (aux psum), reciprocal, and the
                ctxT multiply, all reading the SBUF evacuations."""
                nsl = slice(n * 512, (n + 1) * 512)
                rep = {}
                fillers = []

                def mkrep(bb):
                    def f():
                        rep[bb] = auxpool.tile(
                            [128, 512], F32, tag="p3", name="rp"
                        )
                        nc.tensor.matmul(
                            rep[bb][:], sel8[:, bb, :], rs[:],
                            start=True, stop=True,
                        )
                    return f

                def mknorm(bb):
                    def f():
                        ri = reppool.tile([128, 512], F32, tag="ri", name="ri")
                        scr = reppool.tile([128, 512], F32, tag="scr", name="scr")
                        nc.vector.reciprocal_approx_accurate(
                            ri[:], rep[bb][:], scr[:]
                        )
                        nc.vector.tensor_mul(
                            ctxT[:, p, bb, nsl], avsb[bb][:], ri[:]
                        )
                    return f

                for bb in range(BPC):
                    fillers.append(mkrep(bb))
                    fillers.append(mknorm(bb))
                return fillers

            def make_p3_fillers(n, pools):
                """Out-projection chunks for lq range n (needs ctxT both
                pairs). `pools` is a list of (pool, tag) cycled per chunk.
                Each chunk is TWO closures (matmul / evacuate) so the psum
                slot wait never blocks the PE queue for long."""
                fillers = []

                def mk(bb, msl, osl, pool, tag):
                    st = {}

                    def fmm():
                        po = pool.tile([128, 512], F32, tag=tag, name="po")
                        for kc in range(HDC):
                            nc.tensor.matmul(
                                po[:], ctxT[:, kc, bb, msl], wo_s[:, kc, osl],
                                start=(kc == 0), stop=(kc == HDC - 1),
                            )
                        st["po"] = po

                    def fev():
                        ot = opool.tile([128, 512], BF16, tag="ot", name="ot")
                        nc.vector.tensor_copy(ot[:], st["po"][:])
                        nc.sync.dma_start(out_d[bb, msl, osl], ot[:])

                    return [fmm, fev]

                idx = 0
                for bb in range(BPC):
                    for j in range(4):
                        m0 = n * 512 + j * 128
                        for nn in range(2):
                            pool, tag = pools[idx % len(pools)]
                            idx += 1
                            fillers.extend(
                                mk(bb, slice(m0, m0 + 128),
                                   slice(nn * 512, (nn + 1) * 512), pool, tag)
                            )
                return fillers

            def make_p1_fillers(pair):
                fillers = []
                for b in range(BPC):
                    for c in range(NLC):
                        xq_t = xpool2.tile([128, KSUB, LCH], BF16, tag="xq")
                        xk_t = xpool2.tile([128, KSUB, LCH], BF16, tag="xk")
                        fillers.extend(
                            p1_chunk_ops(pair, b, c, xq_t, xk_t, auxpool, "p3")
                        )
                return fillers

            # global eb prefetch: DMAs cross block seams (depth 3)
            blk_order = [(0, nn) for nn in range(NQ)] + [
                (1, nn) for nn in range(NQ)
            ]
            eb_sched = [
                (pp, nn, ii) for (pp, nn) in blk_order for ii in range(NI)
            ]
            ebts = {}

            def eb_fetch_g(g):
                if g >= len(eb_sched):
                    return
                pp, nn, ii = eb_sched[g]
                ebt = ebpool.tile([128, 2, 512], BF16, tag="eb", name="ebt")
                nc.sync.dma_start(
                    ebt[:],
                    eb_d[
                        2 * pp:2 * pp + 2,
                        ii * 128:(ii + 1) * 128,
                        nn * 512:(nn + 1) * 512,
                    ].rearrange("h p q -> p h q"),
                )
                ebts[g] = ebt

            def block(p, n, carry, relaxed):
                """One attention block: pair p, lq chunk n.

                `carry`: ordered closures from the previous block — its
                leftover av/den matmul groups followed by the psum->SBUF
                evacuations — run 2 per i from i=0 (all done by ~i=3, so
                this block's dummy-clears at i=AVD don't stall the PE).
                `relaxed`: normalization pieces + P1/P3 chunks, run in the
                remaining slack.

                Returns (carry_out, norm_fillers) for the next block."""
                nsl = slice(n * 512, (n + 1) * 512)
                avs = [
                    avpool.tile([128, 512], F32, tag="av", name="av")
                    for _ in range(BPC)
                ]
                avsb = [
                    avsbpool.tile([128, 512], F32, tag="avsb", name="avsb")
                    for _ in range(BPC)
                ]
                den_ps = denpool.tile([128, 512], F32, tag="den", name="den")
                et2s = {}
                cq = list(carry)
                rq = list(relaxed)
                nrelax = (len(rq) + NI - 3) // (NI - 2) if rq else 0

                def emit_avden(j):
                    first = j == 0
                    last = j == NI - 1
                    if first:
                        # pre-clear av/den banks so the real matmuls can all
                        # use start=False (no first_mm bit-clear races)
                        for bb in range(BPC):
                            nc.tensor.matmul(
                                avs[bb][:], zero_w[:], zrow[:],
                                start=True, stop=False, skip_group_check=True,
                            )
                        nc.tensor.matmul(
                            den_ps[:], zero_w[:], zrow[:],
                            start=True, stop=False, skip_group_check=True,
                        )
                    for bb in range(BPC):
                        e2 = et2s[(j, bb)]
                        for hl in range(2):
                            nc.tensor.matmul(
                                avs[bb][hl * 64:(hl + 1) * 64, :],
                                v[:, j, bb, 2 * p + hl, :],
                                e2[:, hl, :],
                                start=False, stop=last,
                                skip_group_check=True,
                                tile_position=(0, hl * 64),
                            )
                    for bb in range(BPC):
                        e2 = et2s[(j, bb)]
                        for hl in range(2):
                            r = 32 * (2 * bb + hl)
                            nc.tensor.matmul(
                                den_ps[r:r + 1, :],
                                ones_m1[:],
                                e2[:, hl, :],
                                start=False, stop=last,
                                skip_group_check=True,
                                tile_position=(0, r),
                            )
                    for bb in range(BPC):
                        del et2s[(j, bb)]

                gbase = blk_order.index((p, n)) * NI
                if gbase == 0:
                    for g in range(3):
                        eb_fetch_g(g)
                for i in range(NI):
                    isl = slice(i * 128, (i + 1) * 128)
                    eb_fetch_g(gbase + i + 3)
                    ebt = ebts.pop(gbase + i)
                    for bb in range(BPC):
                        lg = lgpool.tile([128, 2, 512], F32, tag="lg", name="lg")
                        nc.tensor.matmul(
                            lg[:, 0, :], kT[0:64, p, bb, isl],
                            qT[0:64, p, bb, nsl],
                            start=True, stop=True, tile_position=(0, 0),
                        )
                        nc.tensor.matmul(
                            lg[:, 1, :], kT[64:128, p, bb, isl],
                            qT[64:128, p, bb, nsl],
                            start=True, stop=True, tile_position=(64, 0),
                        )
                        et = etpool.tile([128, 2, 512], BF16, tag="et", name="et")
                        nc.scalar.activation(et[:], lg[:], AF.Exp)
                        e2 = et2pool.tile([128, 2, 512], BF16, tag="et2", name="e2")
                        if GPS_MUL and bb == 1:
                            nc.gpsimd.tensor_mul(e2[:], et[:], ebt[:])
                        else:
                            nc.vector.tensor_mul(e2[:], et[:], ebt[:])
                        et2s[(i, bb)] = e2
                    if i >= AVD:
                        emit_avden(i - AVD)
                    for _ in range(2):
                        if cq:
                            cq.pop(0)()
                    if not cq:
                        for _ in range(nrelax):
                            if rq:
                                rq.pop(0)()
                while cq:
                    cq.pop(0)()
                while rq:
                    rq.pop(0)()
                carry_out = [
                    (lambda j=j: emit_avden(j)) for j in range(NI - AVD, NI)
                ] + make_evac_fillers(avs, den_ps, avsb)
                return carry_out, make_norm_fillers(p, n, avsb)

            def flush(carry):
                for f in carry:
                    f()

            # phase B: pair-0 attention, pair-1 projections as fillers
            p1f = make_p1_fillers(1)
            cs = (len(p1f) + NQ - 1) // NQ
            carry, norm_f = [], []
            for n in range(NQ):
                carry, norm_f = block(
                    0, n, carry, norm_f + p1f[n * cs:(n + 1) * cs]
                )
            # phase C: pair-1 attention, P3 of lq range n-1 as fillers
            for n in range(NQ):
                p3_f = (
                    make_p3_fillers(n - 1, [(auxpool, "p3")]) if n > 0 else []
                )
                carry, norm_f = block(1, n, carry, norm_f + p3_f)
            flush(carry)
            for f in norm_f:
                f()
            for f in make_p3_fillers(
                NQ - 1,
                [(auxpool, "p3"), (lgpool, "lg"), (denpool, "den")],
            ):
                f()

    nc.compile()
    return nc


def make_in_maps(inputs_q, inputs_kv, bias, wq, bq, wk, bk, wv, bv, wo, bo):
    inputs_q = np.asarray(inputs_q, np.float32)
    inputs_kv = np.asarray(inputs_kv, np.float32)
    bias = np.asarray(bias, np.float32)
    wq = np.asarray(wq, np.float32).reshape(D, H * DH)
    wk = np.asarray(wk, np.float32).reshape(D, H * DH)
    wv = np.asarray(wv, np.float32).reshape(D, H * DH)
    bq = np.asarray(bq, np.float32).reshape(H * DH)
    bk = np.asarray(bk, np.float32).reshape(H * DH)
    bv = np.asarray(bv, np.float32).reshape(H * DH)
    wo = np.asarray(wo, np.float32).reshape(H * DH, D)
    bo = np.asarray(bo, np.float32)

    # fold the 1/sqrt(head_dim) query scaling into wq/bq
    s = 1.0 / np.sqrt(DH)
    wq = wq * s
    bq = bq * s

    # host-side layout marshalling for the chosen sharding
    xqT = np.ascontiguousarray(inputs_q.transpose(0, 2, 1)).astype(
        ml_dtypes.bfloat16
    )
    xkT = np.ascontiguousarray(inputs_kv.transpose(0, 2, 1)).astype(
        ml_dtypes.bfloat16
    )
    # exp(bias), transposed to [H, lk, lq], bf16
    ebT = np.exp(bias[0].transpose(0, 2, 1)).astype(ml_dtypes.bfloat16)

    in_maps = []
    for bg in range(NB):
        bsl = slice(bg * BPC, (bg + 1) * BPC)
        for hg in range(NH):
            hsl = slice(hg * HPC, (hg + 1) * HPC)
            csl = slice(hg * HD, (hg + 1) * HD)
            in_maps.append(
                {
                    "xq_t": xqT[bsl],
                    "xk_t": xkT[bsl],
                    "eb_t": np.ascontiguousarray(ebT[hsl]),
                    "wq": np.ascontiguousarray(wq[:, csl]).astype(ml_dtypes.bfloat16),
                    "wk": np.ascontiguousarray(wk[:, csl]).astype(ml_dtypes.bfloat16),
                    "wv": np.ascontiguousarray(wv[:, csl]).astype(ml_dtypes.bfloat16),
                    "wo": np.ascontiguousarray(wo[csl, :]).astype(ml_dtypes.bfloat16),
                    "bq": np.ascontiguousarray(bq[csl]),
                    "bk": np.ascontiguousarray(bk[csl]),
                    "bv": np.ascontiguousarray(bv[csl]).astype(ml_dtypes.bfloat16),
                }
            )
    return in_maps


def assemble(results, bo):
    out = np.zeros((B, L, D), np.float32)
    for bg in range(NB):
        for hg in range(NH):
            out[bg * BPC:(bg + 1) * BPC] += results[bg * NH + hg][
                "out_part"
            ].astype(np.float32)
    out += np.asarray(bo, np.float32)
    return out


def get_nc():
    if "nc" not in _CACHED:
        _CACHED["nc"] = _build_bass()
    return _CACHED["nc"]


def kernel(inputs_q, inputs_kv, bias, wq, bq, wk, bk, wv, bv, wo, bo):
    in_maps = make_in_maps(
        inputs_q, inputs_kv, bias, wq, bq, wk, bk, wv, bv, wo, bo
    )
    res = bass_utils.run_bass_kernel_spmd(
        get_nc(), in_maps, core_ids=list(range(8))
    )
    return assemble(res.results, bo)
